# revision 43
# baseline (speedup 1.0000x reference)
"""BiLSTM classifier on 8 trn2 cores.

Sharding: 2 direction-groups x 4-way batch split (B_local=16).
Cores 0-3: forward direction, batches [0:16),[16:32),[32:48),[48:64).
Cores 4-7: backward direction, same batch slices, with time-reversed
inputs (a backward scan over x == forward scan over reversed x; the
masked SUM pooling is order-invariant so no un-reversal is needed).

Per-core program (identical SPMD program, different inputs):
  phase 1: embedding gather (indirect DMA) + PE transpose -> x_T,
           input projection pre = W_ih^T-augmented @ [x;1] (bias folded
           as an extra ones-feature row), staged to SBUF pre_all
           in per-step [128, (X, hf, b)] layout via PSUM->SBUF copy.
  phase 2: 256-step LSTM scan, gate-partition layout [128, (X,hf,b)],
           fp16 recurrent matmuls, fp32 cell state.
  phase 3: masked mean pool (mask broadcast via ones-matmul) + half
           classifier -> partial logits [3, 16].
Host sums fwd/bwd partial logits (b_c folded into the fwd partial).

The 256-step scan is latency-bound (~1.9us/step critical chain:
matmul -> sigmoid -> tanh(g) -> u -> c -> tanh(c) -> h).  All
off-chain work (PSUM->SBUF staging, mask broadcast, pooling partial
sums) runs on the otherwise-idle GPSIMD engine so it never delays the
chain's Act/DVE ops.

Gate order within a step tile: X in (i, f, o, g), so sigmoid covers
cols 0:96 in one op and tanh(g) covers 96:128.
"""

import os
from contextlib import ExitStack

import numpy as np

import concourse.bass as bass
import concourse.tile as tile
from concourse.tile_rust import add_dep_helper
from concourse import bacc, mybir
from concourse import masks as cmasks
from concourse.bass_utils import run_bass_kernel_spmd

F32 = mybir.dt.float32
F16 = mybir.dt.float16
I32 = mybir.dt.int32
AF = mybir.ActivationFunctionType
OP = mybir.AluOpType

V, E, H, C = 50000, 300, 256, 3
B = 64
NCORES = 8
BL = 16          # batch per core
HB = 2 * BL      # (hf, b) folded free width = 32
G4 = 4 * H       # 1024 gate rows
# permutation of pytorch gate-row order (i,f,g,o) -> kernel order (i,f,o,g)
GATE_PERM = np.r_[0:256, 256:512, 768:1024, 512:768]


# ---------------------------------------------------------------- host prep

def prep_in_maps(input_ids, attention_mask, emb, W_ih_f, W_hh_f, b_ih_f, b_hh_f,
                 W_ih_b, W_hh_b, b_ih_b, b_hh_b, W_c, b_c, T):
    emb_f16 = np.ascontiguousarray(np.asarray(emb, np.float16))
    in_maps = []
    for core in range(NCORES):
        d = core // 4          # 0 fwd, 1 bwd
        bs = slice((core % 4) * BL, (core % 4 + 1) * BL)
        ids = np.asarray(input_ids[bs], np.int32)[:, :T]
        msk = np.asarray(attention_mask[bs], np.float32)[:, :T]
        if d == 1:
            ids = ids[:, ::-1]
            msk = msk[:, ::-1]
        # t-major token order, [T*BL] -> lane-major [128, T*BL/128]
        ids_tb = np.ascontiguousarray(ids.T).reshape(-1)
        ids_in = np.ascontiguousarray(ids_tb.reshape(-1, 128).T)
        # maskrow[0, t*32 + hf*16 + b] = msk[b, t]
        mT = np.ascontiguousarray(msk.T)                      # [T, BL]
        maskrow = np.ascontiguousarray(
            np.stack([mT, mT], axis=1).reshape(1, T * HB))
        maskrow16 = maskrow.astype(np.float16)

        W_ih = (W_ih_f, W_ih_b)[d]
        W_hh = (W_hh_f, W_hh_b)[d]
        bias = (np.asarray(b_ih_f) + np.asarray(b_hh_f),
                np.asarray(b_ih_b) + np.asarray(b_hh_b))[d]
        W_ihp = np.asarray(W_ih, np.float32)[GATE_PERM].copy()  # [1024, 300]
        biasp = np.asarray(bias, np.float32)[GATE_PERM].copy()  # [1024]
        w_ihT = np.ascontiguousarray(
            np.concatenate([W_ihp.T, biasp[None, :]], 0).astype(np.float16))
        w_hhT = np.ascontiguousarray(
            np.asarray(W_hh, np.float32)[GATE_PERM].T.astype(np.float16))
        w_cT = np.ascontiguousarray(
            np.asarray(W_c, np.float32)[:, d * H:(d + 1) * H].T)  # [256, 3]
        bc_eff = (np.asarray(b_c, np.float32).reshape(3, 1) if d == 0
                  else np.zeros((3, 1), np.float32))
        in_maps.append({
            "ids": ids_in,
            "maskrow": maskrow16,
            "maskT2": maskrow.reshape(T, HB).astype(np.float32),
            "w_ihT": w_ihT,
            "w_hhT": w_hhT,
            "w_cT": w_cT,
            "bc": bc_eff,
            "emb": emb_f16,
        })
    return in_maps


def assemble(results):
    logits = np.zeros((B, C), np.float32)
    for core in range(NCORES):
        bs = slice((core % 4) * BL, (core % 4 + 1) * BL)
        logits[bs] += results[core]["out"].T
    return logits


# ---------------------------------------------------------------- kernel

def build_nc(T=256, debug=False):
    nc = bacc.Bacc("TRN2", target_bir_lowering=False, debug=debug,
                   num_devices=NCORES)
    ntok = T * BL
    NTT = ntok // 128             # 128-token tiles (32)

    ids_ap = nc.dram_tensor("ids", [128, NTT], I32, kind="ExternalInput").ap()
    maskrow_ap = nc.dram_tensor("maskrow", [1, T * HB], F16, kind="ExternalInput").ap()
    maskT2_ap = nc.dram_tensor("maskT2", [T, HB], F32, kind="ExternalInput").ap()
    w_ihT_ap = nc.dram_tensor("w_ihT", [E + 1, G4], F16, kind="ExternalInput").ap()
    w_hhT_ap = nc.dram_tensor("w_hhT", [H, G4], F16, kind="ExternalInput").ap()
    w_cT_ap = nc.dram_tensor("w_cT", [H, C], F32, kind="ExternalInput").ap()
    bc_ap = nc.dram_tensor("bc", [C, 1], F32, kind="ExternalInput").ap()
    emb_ap = nc.dram_tensor("emb", [V, E], F16, kind="ExternalInput").ap()
    out_ap = nc.dram_tensor("out", [C, BL], F32, kind="ExternalOutput").ap()

    EK = (128, 128, 44)           # E k-tile sizes
    EO = (0, 128, 256)
    BC = BL                       # batch cols per scan step tile

    with tile.TileContext(nc) as tc:
        with ExitStack() as octx:
            persist = octx.enter_context(tc.tile_pool(name="persist", bufs=1))
            hs = persist.tile([128, (T + 1) * HB], F16, tag="hs")
            idx_all = persist.tile([128, NTT], I32, tag="idx_all")
            wihA = persist.tile([128, 2 * G4], F16, tag="wihA")
            wih2 = persist.tile([EK[2], G4], F16, tag="wih2")
            wbias = persist.tile([1, G4], F16, tag="wbias")
            whhA = persist.tile([128, 2 * G4], F16, tag="whhA")
            ident = persist.tile([128, 128], F32, tag="ident")
            ident16 = persist.tile([128, 128], F16, tag="ident16")
            wcA = persist.tile([128, 2 * C], F32, tag="wcA")
            bc_t = persist.tile([C, 1], F32, tag="bc")
            c0 = persist.tile([128, HB], F32, tag="c0")
            mb = persist.tile([128, T * HB], F16, tag="mb")
            mrow = persist.tile([1, T * HB], F16, tag="mrow")
            ones = persist.tile([1, 128], F16, tag="ones")
            ones512 = persist.tile([1, 512], F16, tag="ones512")
            ones128 = persist.tile([128, 128], F32, tag="ones128")
            nkt = (T + 127) // 128
            mt2 = [persist.tile([min(128, T - 128 * k), HB], F32,
                                tag=f"mt2_{k}", name=f"mt2_{k}")
                   for k in range(nkt)]
            cnt_r = persist.tile([128, HB], F32, tag="cnt_r")

            # critical-path DMAs first (SP queue is in-order, 565ns per issue)
            nc.sync.dma_start(idx_all[:], ids_ap[:])
            nc.sync.dma_start(
                wihA[:].rearrange("p (k c) -> p k c", k=2),
                w_ihT_ap[0:256, :].rearrange("(k p) c -> p k c", k=2))
            nc.sync.dma_start(wih2[:], w_ihT_ap[EO[2]:EO[2] + EK[2], :])
            nc.sync.dma_start(
                whhA[:].rearrange("p (k c) -> p k c", k=2),
                w_hhT_ap[:].rearrange("(k p) c -> p k c", k=2))
            nc.sync.dma_start(mrow[:], maskrow_ap[:])
            nc.sync.dma_start(wbias[:], w_ihT_ap[E:E + 1, :])
            # cold-path DMAs on the Act queue (keeps SP free)
            nc.scalar.dma_start(
                wcA[:].rearrange("p (k c) -> p k c", k=2),
                w_cT_ap[:].rearrange("(k p) c -> p k c", k=2))
            nc.scalar.dma_start(bc_t[:], bc_ap[:])
            for k in range(nkt):
                nc.scalar.dma_start(
                    mt2[k][:], maskT2_ap[128 * k:min(128 * (k + 1), T), :])
            cmasks.make_identity(nc, ident[:])
            cmasks.make_identity(nc, ident16[:])
            nc.vector.memset(c0[:], 0.0)
            nc.vector.memset(hs[:, 0:HB], 0.0)
            nc.vector.memset(ones[:], 1.0)
            nc.vector.memset(ones512[:], 1.0)
            nc.gpsimd.memset(ones128[:], 1.0)

            def wih(k):
                if k < 2:
                    return wihA[:, k * G4:(k + 1) * G4]
                return wih2[:]

            def whh(k):
                return whhA[:, k * G4:(k + 1) * G4]

            with ExitStack() as mp:
                prep = mp.enter_context(tc.tile_pool(name="pre", bufs=4))
                xgp = mp.enter_context(tc.tile_pool(name="xg", bufs=8))
                xtp = mp.enter_context(tc.tile_pool(name="xt", bufs=2))
                tpp = mp.enter_context(
                    tc.tile_pool(name="tp", bufs=2, space="PSUM"))
                prp = mp.enter_context(
                    tc.tile_pool(name="prj", bufs=2, space="PSUM"))
                gp = mp.enter_context(
                    tc.tile_pool(name="gates", bufs=2, space="PSUM"))
                sp = mp.enter_context(tc.tile_pool(name="sig", bufs=3))
                cp = mp.enter_context(tc.tile_pool(name="cell", bufs=3))
                pp_pool = mp.enter_context(tc.tile_pool(name="pool", bufs=1))

                # chunk schedule: small chunks first for fast scan start
                sizes = [8, 8, 16] + [32] * ((T - 32) // 32)
                assert sum(sizes) == T
                starts = [sum(sizes[:i]) for i in range(len(sizes))]
                chunks = list(zip(starts, sizes))
                pre_ch = {}

                gather_tp = {}
                anchors = {"mm": None, "h": None}

                def anch(inst, which):
                    """scheduler-only edge: this prep op may not be scheduled
                    before the current step's chain anchor (last recurrent
                    matmul for PE preps, the h-write for DVE preps)."""
                    a = anchors[which]
                    if a is not None and inst is not None:
                        add_dep_helper(
                            getattr(inst, "ins", inst), getattr(a, "ins", a),
                            sync=False, reason="prep-after-chain")

                def gather_fetch(t0, tt):
                    """gather 128 tokens (8 steps) + PE transpose into PSUM"""
                    g = (t0 * BL) // 128 + tt
                    xg = xgp.tile([128, E], F16, tag="xg", name=f"xg{t0}_{tt}")
                    nc.gpsimd.indirect_dma_start(
                        out=xg[:], out_offset=None, in_=emb_ap[:],
                        in_offset=bass.IndirectOffsetOnAxis(
                            ap=idx_all[:, g:g + 1], axis=0),
                    )
                    tp = tpp.tile([128, 3 * 128], F16, tag="tp",
                                  name=f"tp{t0}_{tt}")
                    for k in range(3):
                        anch(nc.tensor.transpose(
                            tp[:EK[k], bass.ts(k, 128)],
                            xg[:, EO[k]:EO[k] + EK[k]], ident16[:]), "mm")
                    gather_tp[(t0, tt)] = tp

                def gather_copy(t0, tt):
                    """PSUM->SBUF xt copies (deferred past the h write)"""
                    xt = pre_ch[t0]["xt"]
                    tp = gather_tp.pop((t0, tt))
                    for k in range(3):
                        anch(nc.vector.tensor_copy(
                            xt[k][:EK[k], bass.ts(tt, 128)],
                            tp[:EK[k], bass.ts(k, 128)]), "h")

                proj_pj = {}

                def proj_mm(t0, ns, m):
                    """project m-tile m for chunk at t0 (ns steps).

                    N split in halves so a ready proj matmul at the PE queue
                    head blocks a scan matmul by <=107ns."""
                    xt = pre_ch[t0]["xt"]
                    N = ns * BL
                    pj = prp.tile([128, 512], F32, tag="prj", name=f"pj{t0}_{m}")
                    proj_pj[(t0, m)] = pj
                    for c0 in range(0, N, 256):
                        cw = min(256, N - c0)
                        for k in range(3):
                            anch(nc.tensor.matmul(
                                pj[:, c0:c0 + cw],
                                wih(k)[:, bass.ts(m, 128)],
                                xt[k][:, c0:c0 + cw],
                                start=(k == 0), stop=False), "mm")
                        anch(nc.tensor.matmul(
                            pj[:, c0:c0 + cw], wbias[:, bass.ts(m, 128)],
                            ones512[:, :cw], start=False, stop=True), "mm")

                def proj_stage(t0, ns, m, half=None):
                    """stage PSUM->SBUF in [128,128] pieces (short DVE ops so
                    the greedy scheduler can't block a chain op for long)"""
                    N = ns * BL
                    pj = proj_pj[(t0, m)]
                    lo, hi = 0, N
                    if half == 0:
                        hi = N // 2
                    elif half == 1:
                        lo = N // 2
                        proj_pj.pop((t0, m))
                    else:
                        proj_pj.pop((t0, m))
                    X, hf = m // 2, m % 2
                    dst = pre_ch[t0]["pre"][:].rearrange(
                        "p (t x) -> p t x", x=128)[
                        :, :, X * 32 + hf * 16:X * 32 + hf * 16 + 16]
                    for c0 in range(lo, hi, 128):
                        cw = min(128, hi - c0)
                        anch(nc.vector.tensor_copy(
                            dst[:, c0 // 16:(c0 + cw) // 16],
                            pj[:, c0:c0 + cw].rearrange(
                                "p (t b) -> p t b", b=16)), "h")

                def chunk_work(ci):
                    """typed work items ('pe'|'dve', closure) for chunk ci.

                    Items are popped strictly in order: 'pe' items at the
                    post-matmul fence (PE idle window), 'dve' items at the
                    post-h fence (DVE idle window)."""
                    t0, ns = chunks[ci]
                    ntt = ns * BL // 128
                    pre = prep.tile([128, ns * 128], F16, tag="pre",
                                    name=f"pre{ci}")
                    xt = [xtp.tile([EK[k], ns * BL], F16, tag=f"xt{k}",
                                   name=f"xt{k}_{ci}") for k in range(3)]
                    pre_ch[t0] = {"pre": pre, "xt": xt}
                    items = []
                    for tt in range(ntt):
                        items.append(("pe", lambda tt=tt: gather_fetch(t0, tt)))
                        items.append(("dve", lambda tt=tt: gather_copy(t0, tt)))
                    for m in range(8):
                        items.append(("pe", lambda m=m: proj_mm(t0, ns, m)))
                        if ns >= 32:
                            items.append(
                                ("dve", lambda m=m: proj_stage(t0, ns, m, 0)))
                            items.append(
                                ("dve", lambda m=m: proj_stage(t0, ns, m, 1)))
                        else:
                            items.append(
                                ("dve", lambda m=m: proj_stage(t0, ns, m)))
                    return items

                built_j = [0]
                mb_pb = {}

                def mb_mm(j):
                    pb = prp.tile([128, 512], F32, tag="prj", name=f"pb{j}")
                    mb_pb[j] = pb
                    for c0 in range(0, 512, 256):
                        anch(nc.tensor.matmul(
                            pb[:, c0:c0 + 256],
                            ones[:], mrow[:, j * 512 + c0:j * 512 + c0 + 256],
                            start=True, stop=True), "mm")

                def mb_copy(j):
                    pb = mb_pb.pop(j)
                    for c0 in range(0, 512, 128):
                        anch(nc.vector.tensor_copy(
                            mb[:, j * 512 + c0:j * 512 + c0 + 128],
                            pb[:, c0:c0 + 128]), "h")

                def mb_items(t1):
                    items = []
                    while built_j[0] * 512 < t1 * HB:
                        j = built_j[0]
                        items.append(("pe", lambda j=j: mb_mm(j)))
                        items.append(("dve", lambda j=j: mb_copy(j)))
                        built_j[0] += 1
                    return items

                cnt_pj = {}

                def count_mm():
                    cntp = prp.tile([128, HB], F32, tag="prj", name="cntp")
                    cnt_pj[0] = cntp
                    for k in range(nkt):
                        anch(nc.tensor.matmul(
                            cntp[:], ones128[:mt2[k].shape[0], :],
                            mt2[k][:], start=(k == 0),
                            stop=(k == nkt - 1)), "mm")

                def count_fin():
                    """masked-count reciprocal (for mean pooling), off-path"""
                    cntp = cnt_pj.pop(0)
                    cnt = pp_pool.tile([128, HB], F32, tag="cnt")
                    anch(nc.vector.tensor_scalar_max(cnt[:], cntp[:], 1e-9), "h")
                    anch(nc.vector.reciprocal(cnt_r[:], cnt[:]), "h")

                st = {"c": c0[:, 0:2 * BC], "sig": None, "sigo": None, "cn": None}

                def front(t):
                    """gate matmuls in 2 psum banks + sig/tanh + c update.

                    bank A holds (i, f), bank B holds (o, g): sigmoid(i,f)
                    issues after only 8 of the 16 recurrent matmuls."""
                    ck = max(i for i, (s, _) in enumerate(chunks) if s <= t)
                    t0 = chunks[ck][0]
                    pre_t = pre_ch[t0]["pre"][:, bass.ts(t - t0, 128)]
                    pa = gp.tile([128, 4 * BC], F32, tag="ga", name="ga")
                    pb = gp.tile([128, 4 * BC], F32, tag="gb", name="gb")
                    nc.tensor.matmul(pa[:], ident16[:], pre_t[:, 0:4 * BC],
                                     start=True, stop=False)
                    nc.tensor.matmul(pb[:], ident16[:], pre_t[:, 4 * BC:8 * BC],
                                     start=True, stop=False)
                    for bank, x in [(pa, 0), (pa, 1), (pb, 2), (pb, 3)]:
                        for hf in range(2):
                            for k in range(2):
                                bank_last = (x % 2 == 1 and hf == 1 and k == 1)
                                mi = nc.tensor.matmul(
                                    bank[:, (x % 2) * 2 * BC + hf * BC:
                                         (x % 2) * 2 * BC + (hf + 1) * BC],
                                    whh(k)[:, x * 256 + hf * 128:x * 256 + (hf + 1) * 128],
                                    hs[:, t * HB + k * 16:t * HB + k * 16 + BC],
                                    start=False, stop=bank_last)
                    anchors["mm"] = mi
                    sig = sp.tile([128, 4 * BC], F16, tag="sig", name="sig")
                    nc.scalar.activation(sig[:], pa[:], AF.Sigmoid)
                    tg = sp.tile([128, 2 * BC], F16, tag="tg", name="tg")
                    nc.scalar.activation(tg[:], pb[:, 2 * BC:4 * BC], AF.Tanh)
                    sigo = sp.tile([128, 2 * BC], F16, tag="sigo", name="sigo")
                    nc.scalar.activation(sigo[:], pb[:, 0:2 * BC], AF.Sigmoid)
                    v = cp.tile([128, 2 * BC], F32, tag="v", name="v")
                    nc.vector.tensor_tensor(v[:], sig[:, 2 * BC:4 * BC],
                                            st["c"], OP.mult)
                    u = cp.tile([128, 2 * BC], F16, tag="u", name="u")
                    nc.vector.tensor_tensor(u[:], sig[:, 0:2 * BC], tg[:], OP.mult)
                    cn = cp.tile([128, 2 * BC], F32, tag="c", name="c")
                    nc.vector.tensor_tensor(cn[:], u[:], v[:], OP.add)
                    st["sig"], st["sigo"], st["cn"] = sig, sigo, cn

                def tail(t):
                    """h = sig_o * tanh(c)"""
                    sigo, cn = st["sigo"], st["cn"]
                    thc = sp.tile([128, 2 * BC], F16, tag="thc", name="thc")
                    nc.scalar.activation(thc[:], cn[:], AF.Tanh)
                    anchors["h"] = nc.vector.tensor_tensor(
                        hs[:, (t + 1) * HB:(t + 2) * HB],
                        sigo[:], thc[:], OP.mult)
                    st["c"] = cn

                PP = 16                     # steps per pooling piece
                parts = []

                def pool_piece(t0, n=PP):
                    """masked partial sum of h over steps [t0, t0+n)"""
                    mk = pp_pool.tile([128, PP * HB], F16, tag="mk",
                                      name=f"mk{t0}", bufs=2)
                    nc.gpsimd.tensor_tensor(
                        mk[:, :n * HB], hs[:, (t0 + 1) * HB:(t0 + n + 1) * HB],
                        mb[:, t0 * HB:(t0 + n) * HB], OP.mult)
                    # reduce over t as a binary tree of gpsimd adds
                    # (gpsimd tensor_reduce can't reduce the free axis)
                    cur = mk
                    w = n * HB
                    while w > HB:
                        w //= 2
                        nxt = pp_pool.tile([128, w], F32, tag=f"mkr{w}",
                                           name=f"mkr{w}_{t0}", bufs=2)
                        nc.gpsimd.tensor_tensor(
                            nxt[:], cur[:, 0:w], cur[:, w:2 * w], OP.add)
                        cur = nxt
                    parts.append(cur)
                    if len(parts) >= 2:
                        a, b = parts.pop(), parts.pop()
                        s = pp_pool.tile([128, HB], F32, tag="psum",
                                         name=f"ps{t0}", bufs=2)
                        nc.gpsimd.tensor_tensor(s[:], a[:], b[:], OP.add)
                        parts.append(s)

                # ---------------- interleaved schedule
                #
                # Single strictly-ordered work queue, popped only from the
                # head. 'pe' items (gather DMA + transposes, proj matmuls) are
                # issued behind a scheduler fence placed right after front(t),
                # so they fill the long PE-idle stretch of the step without
                # ever being schedulable ahead of the recurrent matmuls. 'dve'
                # items (PSUM->SBUF copies) are issued behind a fence placed
                # after tail(t), so they fill the post-h DVE-idle window and
                # can never head-block a chain op. Popping only from the head
                # keeps issue order = queue order, which the PSUM pool slab
                # cycling (prj/tp tags) relies on.
                from collections import deque
                work = deque()
                for kind, fn in (chunk_work(0) + mb_items(16) + chunk_work(1)):
                    fn()
                work.extend(chunk_work(2))
                work.append(("pe", count_mm))
                work.append(("dve", count_fin))
                next_chunk = 3
                for ci in range(len(chunks)):
                    t0, ns = chunks[ci]
                    if next_chunk < len(chunks):
                        work.extend(chunk_work(next_chunk))
                        next_chunk += 1
                    work.extendleft(reversed(mb_items(t0 + ns)))
                    for t in range(t0, t0 + ns):
                        with tc.high_priority():
                            front(t)
                            tail(t)
                        if t >= PP + PP // 2 and (t - PP // 2) % PP == 0 \
                                and t - PP - PP // 2 < T - PP:
                            pool_piece(t - PP - PP // 2)
                        if t == T - 5:
                            pool_piece(T - PP, PP // 2)
                        for _ in range(4):
                            if work:
                                work.popleft()[1]()
                assert not work, f"{len(work)} work items unissued"
                pool_piece(T - PP // 2, PP // 2)

                # ---------------- tail: pooled -> logits
                while len(parts) > 1:
                    a, b = parts.pop(), parts.pop()
                    s = pp_pool.tile([128, HB], F32, tag="psum",
                                     name=f"fin{len(parts)}", bufs=2)
                    nc.vector.tensor_tensor(s[:], a[:], b[:], OP.add)
                    parts.append(s)
                pooled = parts[0]

                pn = pp_pool.tile([128, HB], F32, tag="pn")
                nc.vector.tensor_tensor(pn[:], pooled[:], cnt_r[:], OP.mult)
                lg = prp.tile([C, BL], F32, tag="prj", name="lg")
                for k in range(2):
                    nc.tensor.matmul(lg[:], wcA[:, k * C:(k + 1) * C],
                                     pn[:, k * BL:(k + 1) * BL],
                                     start=(k == 0), stop=(k == 1))
                ot = pp_pool.tile([C, BL], F32, tag="ot")
                nc.scalar.activation(ot[:], lg[:], AF.Identity, bias=bc_t[:])
                nc.sync.dma_start(out_ap[:], ot[:])

    nc.compile()
    return nc


# ---------------------------------------------------------------- entry

_NC_CACHE = {}


def kernel(**inputs) -> np.ndarray:
    """BiLSTM classifier forward on 8 trn2 NeuronCores.

    Takes the full unsharded inputs (as produced by setup_inputs()), runs
    the SPMD bass kernel on cores 0-7, returns full [64, 3] f32 logits.
    """
    T = 256
    if T not in _NC_CACHE:
        _NC_CACHE[T] = build_nc(T=T)
    nc = _NC_CACHE[T]
    np_inputs = {k: np.asarray(v) for k, v in inputs.items()}
    in_maps = prep_in_maps(T=T, **np_inputs)
    res = run_bass_kernel_spmd(nc, in_maps, list(range(NCORES)))
    return assemble(res.results)


# revision 49
# speedup vs baseline: 1.0410x; 1.0410x over previous
"""BiLSTM classifier on 8 trn2 cores.

Sharding: 2 direction-groups x 4-way batch split (B_local=16).
Cores 0-3: forward direction, batches [0:16),[16:32),[32:48),[48:64).
Cores 4-7: backward direction, same batch slices, with time-reversed
inputs (a backward scan over x == forward scan over reversed x; the
masked SUM pooling is order-invariant so no un-reversal is needed).

Per-core program (identical SPMD program, different inputs):
  phase 1: embedding gather (indirect DMA) + PE transpose -> x_T,
           input projection pre = W_ih^T-augmented @ [x;1] (bias folded
           as an extra ones-feature row), staged to SBUF pre_all
           in per-step [128, (X, hf, b)] layout via PSUM->SBUF copy.
  phase 2: 256-step LSTM scan, gate-partition layout [128, (X,hf,b)],
           fp16 recurrent matmuls, fp32 cell state.
  phase 3: masked mean pool (mask broadcast via ones-matmul) + half
           classifier -> partial logits [3, 16].
Host sums fwd/bwd partial logits (b_c folded into the fwd partial).

The 256-step scan is latency-bound (~1.9us/step critical chain:
matmul -> sigmoid -> tanh(g) -> u -> c -> tanh(c) -> h).  All
off-chain work (PSUM->SBUF staging, mask broadcast, pooling partial
sums) runs on the otherwise-idle GPSIMD engine so it never delays the
chain's Act/DVE ops.

Gate order within a step tile: X in (i, f, o, g), so sigmoid covers
cols 0:96 in one op and tanh(g) covers 96:128.
"""

import os
from contextlib import ExitStack

import numpy as np

import concourse.bass as bass
import concourse.tile as tile
from concourse.tile_rust import add_dep_helper
from concourse import bacc, mybir
from concourse import masks as cmasks
from concourse.bass_utils import run_bass_kernel_spmd

F32 = mybir.dt.float32
F16 = mybir.dt.float16
I32 = mybir.dt.int32
AF = mybir.ActivationFunctionType
OP = mybir.AluOpType

V, E, H, C = 50000, 300, 256, 3
B = 64
NCORES = 8
BL = 16          # batch per core
HB = 2 * BL      # (hf, b) folded free width = 32
G4 = 4 * H       # 1024 gate rows
# permutation of pytorch gate-row order (i,f,g,o) -> kernel order (i,f,o,g)
GATE_PERM = np.r_[0:256, 256:512, 768:1024, 512:768]


# ---------------------------------------------------------------- host prep

def prep_in_maps(input_ids, attention_mask, emb, W_ih_f, W_hh_f, b_ih_f, b_hh_f,
                 W_ih_b, W_hh_b, b_ih_b, b_hh_b, W_c, b_c, T):
    emb_f16 = np.ascontiguousarray(np.asarray(emb, np.float16))
    in_maps = []
    for core in range(NCORES):
        d = core // 4          # 0 fwd, 1 bwd
        bs = slice((core % 4) * BL, (core % 4 + 1) * BL)
        ids = np.asarray(input_ids[bs], np.int32)[:, :T]
        msk = np.asarray(attention_mask[bs], np.float32)[:, :T]
        if d == 1:
            ids = ids[:, ::-1]
            msk = msk[:, ::-1]
        # t-major token order, [T*BL] -> lane-major [128, T*BL/128]
        ids_tb = np.ascontiguousarray(ids.T).reshape(-1)
        ids_in = np.ascontiguousarray(ids_tb.reshape(-1, 128).T)
        # maskrow[0, t*32 + hf*16 + b] = msk[b, t]
        mT = np.ascontiguousarray(msk.T)                      # [T, BL]
        maskrow = np.ascontiguousarray(
            np.stack([mT, mT], axis=1).reshape(1, T * HB))
        maskrow16 = maskrow.astype(np.float16)

        W_ih = (W_ih_f, W_ih_b)[d]
        W_hh = (W_hh_f, W_hh_b)[d]
        bias = (np.asarray(b_ih_f) + np.asarray(b_hh_f),
                np.asarray(b_ih_b) + np.asarray(b_hh_b))[d]
        W_ihp = np.asarray(W_ih, np.float32)[GATE_PERM].copy()  # [1024, 300]
        biasp = np.asarray(bias, np.float32)[GATE_PERM].copy()  # [1024]
        w_ihT = np.ascontiguousarray(
            np.concatenate([W_ihp.T, biasp[None, :]], 0).astype(np.float16))
        w_hhT = np.ascontiguousarray(
            np.asarray(W_hh, np.float32)[GATE_PERM].T.astype(np.float16))
        w_cT = np.ascontiguousarray(
            np.asarray(W_c, np.float32)[:, d * H:(d + 1) * H].T)  # [256, 3]
        bc_eff = (np.asarray(b_c, np.float32).reshape(3, 1) if d == 0
                  else np.zeros((3, 1), np.float32))
        in_maps.append({
            "ids": ids_in,
            "maskrow": maskrow16,
            "maskT2": maskrow.reshape(T, HB).astype(np.float32),
            "w_ihT": w_ihT,
            "w_hhT": w_hhT,
            "w_cT": w_cT,
            "bc": bc_eff,
            "emb": emb_f16,
        })
    return in_maps


def assemble(results):
    logits = np.zeros((B, C), np.float32)
    for core in range(NCORES):
        bs = slice((core % 4) * BL, (core % 4 + 1) * BL)
        logits[bs] += results[core]["out"].T
    return logits


# ---------------------------------------------------------------- kernel

def build_nc(T=256, debug=False):
    nc = bacc.Bacc("TRN2", target_bir_lowering=False, debug=debug,
                   num_devices=NCORES)
    ntok = T * BL
    NTT = ntok // 128             # 128-token tiles (32)

    ids_ap = nc.dram_tensor("ids", [128, NTT], I32, kind="ExternalInput").ap()
    maskrow_ap = nc.dram_tensor("maskrow", [1, T * HB], F16, kind="ExternalInput").ap()
    maskT2_ap = nc.dram_tensor("maskT2", [T, HB], F32, kind="ExternalInput").ap()
    w_ihT_ap = nc.dram_tensor("w_ihT", [E + 1, G4], F16, kind="ExternalInput").ap()
    w_hhT_ap = nc.dram_tensor("w_hhT", [H, G4], F16, kind="ExternalInput").ap()
    w_cT_ap = nc.dram_tensor("w_cT", [H, C], F32, kind="ExternalInput").ap()
    bc_ap = nc.dram_tensor("bc", [C, 1], F32, kind="ExternalInput").ap()
    emb_ap = nc.dram_tensor("emb", [V, E], F16, kind="ExternalInput").ap()
    out_ap = nc.dram_tensor("out", [C, BL], F32, kind="ExternalOutput").ap()

    EK = (128, 128, 44)           # E k-tile sizes
    EO = (0, 128, 256)
    BC = BL                       # batch cols per scan step tile

    with tile.TileContext(nc) as tc:
        with ExitStack() as octx:
            persist = octx.enter_context(tc.tile_pool(name="persist", bufs=1))
            hs = persist.tile([128, (T + 1) * HB], F16, tag="hs")
            idx_all = persist.tile([128, NTT], I32, tag="idx_all")
            wihA = persist.tile([128, 2 * G4], F16, tag="wihA")
            wih2 = persist.tile([EK[2], G4], F16, tag="wih2")
            wbias = persist.tile([1, G4], F16, tag="wbias")
            whhA = persist.tile([128, 2 * G4], F16, tag="whhA")
            ident16 = persist.tile([128, 128], F16, tag="ident16")
            wcA = persist.tile([128, 2 * C], F32, tag="wcA")
            bc_t = persist.tile([C, 1], F32, tag="bc")
            c0 = persist.tile([128, HB], F32, tag="c0")
            mb = persist.tile([128, T * HB], F16, tag="mb")
            mrow = persist.tile([1, T * HB], F16, tag="mrow")
            ones = persist.tile([1, 128], F16, tag="ones")
            ones512 = persist.tile([1, 512], F16, tag="ones512")
            ones128 = persist.tile([128, 128], F32, tag="ones128")
            nkt = (T + 127) // 128
            mt2 = [persist.tile([min(128, T - 128 * k), HB], F32,
                                tag=f"mt2_{k}", name=f"mt2_{k}")
                   for k in range(nkt)]
            cnt_r = persist.tile([128, HB], F32, tag="cnt_r")

            # critical-path DMAs first (SP queue is in-order, 565ns per issue)
            nc.sync.dma_start(idx_all[:], ids_ap[:])
            nc.sync.dma_start(
                wihA[:].rearrange("p (k c) -> p k c", k=2),
                w_ihT_ap[0:256, :].rearrange("(k p) c -> p k c", k=2))
            nc.sync.dma_start(wih2[:], w_ihT_ap[EO[2]:EO[2] + EK[2], :])
            nc.sync.dma_start(
                whhA[:].rearrange("p (k c) -> p k c", k=2),
                w_hhT_ap[:].rearrange("(k p) c -> p k c", k=2))
            nc.sync.dma_start(mrow[:], maskrow_ap[:])
            nc.sync.dma_start(wbias[:], w_ihT_ap[E:E + 1, :])
            # cold-path DMAs on the Act queue (keeps SP free)
            nc.scalar.dma_start(
                wcA[:].rearrange("p (k c) -> p k c", k=2),
                w_cT_ap[:].rearrange("(k p) c -> p k c", k=2))
            nc.scalar.dma_start(bc_t[:], bc_ap[:])
            for k in range(nkt):
                nc.scalar.dma_start(
                    mt2[k][:], maskT2_ap[128 * k:min(128 * (k + 1), T), :])
            cmasks.make_identity(nc, ident16[:])
            nc.vector.memset(c0[:], 0.0)
            nc.vector.memset(hs[:, 0:HB], 0.0)
            nc.vector.memset(ones[:], 1.0)
            nc.vector.memset(ones512[:], 1.0)
            nc.gpsimd.memset(ones128[:], 1.0)

            def wih(k):
                if k < 2:
                    return wihA[:, k * G4:(k + 1) * G4]
                return wih2[:]

            def whh(k):
                return whhA[:, k * G4:(k + 1) * G4]

            with ExitStack() as mp:
                xgp = mp.enter_context(tc.tile_pool(name="xg", bufs=8))
                xtp = mp.enter_context(tc.tile_pool(name="xt", bufs=3))
                tpp = mp.enter_context(
                    tc.tile_pool(name="tp", bufs=2, space="PSUM"))
                prp = mp.enter_context(
                    tc.tile_pool(name="prj", bufs=2, space="PSUM"))
                gp = mp.enter_context(
                    tc.tile_pool(name="gates", bufs=2, space="PSUM"))
                sp = mp.enter_context(tc.tile_pool(name="sig", bufs=3))
                cp = mp.enter_context(tc.tile_pool(name="cell", bufs=3))
                pp_pool = mp.enter_context(tc.tile_pool(name="pool", bufs=1))

                # chunk schedule: small chunks first for fast scan start
                sizes = [8, 8, 16] + [32] * ((T - 32) // 32)
                assert sum(sizes) == T
                starts = [sum(sizes[:i]) for i in range(len(sizes))]
                chunks = list(zip(starts, sizes))
                pre_ch = {}

                gather_tp = {}
                anchors = {"mm": None, "h": None}

                def anch(inst, which):
                    """scheduler-only edge: this prep op may not be scheduled
                    before the current step's chain anchor (last recurrent
                    matmul for PE preps, the h-write for DVE preps)."""
                    a = anchors[which]
                    if a is not None and inst is not None:
                        add_dep_helper(
                            getattr(inst, "ins", inst), getattr(a, "ins", a),
                            sync=False, reason="prep-after-chain")

                def gather_fetch(t0, tt):
                    """gather 128 tokens (8 steps) + PE transpose into PSUM"""
                    g = (t0 * BL) // 128 + tt
                    xg = xgp.tile([128, E], F16, tag="xg", name=f"xg{t0}_{tt}")
                    nc.gpsimd.indirect_dma_start(
                        out=xg[:], out_offset=None, in_=emb_ap[:],
                        in_offset=bass.IndirectOffsetOnAxis(
                            ap=idx_all[:, g:g + 1], axis=0),
                    )
                    tp = tpp.tile([128, 3 * 128], F16, tag="tp",
                                  name=f"tp{t0}_{tt}")
                    for k in range(3):
                        anch(nc.tensor.transpose(
                            tp[:EK[k], bass.ts(k, 128)],
                            xg[:, EO[k]:EO[k] + EK[k]], ident16[:]), "mm")
                    gather_tp[(t0, tt)] = tp

                def gather_copy(t0, tt):
                    """PSUM->SBUF xt copies (deferred past the h write)"""
                    xt = pre_ch[t0]["xt"]
                    tp = gather_tp.pop((t0, tt))
                    for k in range(3):
                        anch(nc.vector.tensor_copy(
                            xt[k][:EK[k], bass.ts(tt, 128)],
                            tp[:EK[k], bass.ts(k, 128)]), "h")

                def chunk_work(ci):
                    """typed work items ('pe'|'dve', closure) for chunk ci:
                    gather fetch+transpose (PE window) and PSUM->SBUF xt
                    copies (post-h DVE window). The input projection itself
                    happens inside front(t), accumulated straight into the
                    gate PSUM banks."""
                    t0, ns = chunks[ci]
                    ntt = ns * BL // 128
                    xt = [xtp.tile([EK[k], ns * BL], F16, tag=f"xt{k}",
                                   name=f"xt{k}_{ci}") for k in range(3)]
                    pre_ch[t0] = {"xt": xt}
                    items = []
                    for tt in range(ntt):
                        items.append(("pe", lambda tt=tt: gather_fetch(t0, tt)))
                        items.append(("dve", lambda tt=tt: gather_copy(t0, tt)))
                    return items

                built_j = [0]
                mb_pb = {}

                def mb_mm(j):
                    pb = prp.tile([128, 512], F32, tag="prj", name=f"pb{j}")
                    mb_pb[j] = pb
                    for c0 in range(0, 512, 256):
                        anch(nc.tensor.matmul(
                            pb[:, c0:c0 + 256],
                            ones[:], mrow[:, j * 512 + c0:j * 512 + c0 + 256],
                            start=True, stop=True), "mm")

                def mb_copy(j):
                    pb = mb_pb.pop(j)
                    for c0 in range(0, 512, 128):
                        anch(nc.vector.tensor_copy(
                            mb[:, j * 512 + c0:j * 512 + c0 + 128],
                            pb[:, c0:c0 + 128]), "h")

                def mb_items(t1):
                    items = []
                    while built_j[0] * 512 < t1 * HB:
                        j = built_j[0]
                        items.append(("pe", lambda j=j: mb_mm(j)))
                        items.append(("dve", lambda j=j: mb_copy(j)))
                        built_j[0] += 1
                    return items

                cnt_pj = {}

                def count_mm():
                    cntp = prp.tile([128, HB], F32, tag="prj", name="cntp")
                    cnt_pj[0] = cntp
                    for k in range(nkt):
                        anch(nc.tensor.matmul(
                            cntp[:], ones128[:mt2[k].shape[0], :],
                            mt2[k][:], start=(k == 0),
                            stop=(k == nkt - 1)), "mm")

                def count_fin():
                    """masked-count reciprocal (for mean pooling), off-path"""
                    cntp = cnt_pj.pop(0)
                    cnt = pp_pool.tile([128, HB], F32, tag="cnt")
                    anch(nc.vector.tensor_scalar_max(cnt[:], cntp[:], 1e-9), "h")
                    anch(nc.vector.reciprocal(cnt_r[:], cnt[:]), "h")

                st = {"c": c0[:, 0:2 * BC], "sig": None, "sigo": None, "cn": None}

                def front(t):
                    """gate matmuls in 2 psum banks + sig/tanh + c update.

                    bank A holds (i, f), bank B holds (o, g). The input
                    projection (W_ih k-tiles + bias) accumulates into the
                    same banks; those matmuls depend only on xt, so they
                    execute early in the step's PE-idle window. sigmoid(i,f)
                    issues after only 8 of the 16 recurrent matmuls."""
                    ck = max(i for i, (s, _) in enumerate(chunks) if s <= t)
                    t0 = chunks[ck][0]
                    xt = pre_ch[t0]["xt"]
                    tb = t - t0
                    pa = gp.tile([128, 4 * BC], F32, tag="ga", name="ga")
                    pb = gp.tile([128, 4 * BC], F32, tag="gb", name="gb")
                    for bank, x in [(pa, 0), (pa, 1), (pb, 2), (pb, 3)]:
                        for hf in range(2):
                            m = x * 2 + hf
                            col = bank[:, (x % 2) * 2 * BC + hf * BC:
                                       (x % 2) * 2 * BC + (hf + 1) * BC]
                            for k in range(3):
                                # exactly one start per bank: PSUM's zero
                                # region is the whole 2KB bank row
                                nc.tensor.matmul(
                                    col, wih(k)[:, bass.ts(m, 128)],
                                    xt[k][:, tb * BL:(tb + 1) * BL],
                                    start=(x % 2 == 0 and hf == 0 and k == 0),
                                    stop=False)
                            nc.tensor.matmul(
                                col, wbias[:, bass.ts(m, 128)],
                                ones512[:, :BL], start=False, stop=False)
                    for bank, x in [(pa, 0), (pa, 1), (pb, 2), (pb, 3)]:
                        for hf in range(2):
                            for k in range(2):
                                bank_last = (x % 2 == 1 and hf == 1 and k == 1)
                                mi = nc.tensor.matmul(
                                    bank[:, (x % 2) * 2 * BC + hf * BC:
                                         (x % 2) * 2 * BC + (hf + 1) * BC],
                                    whh(k)[:, x * 256 + hf * 128:x * 256 + (hf + 1) * 128],
                                    hs[:, t * HB + k * 16:t * HB + k * 16 + BC],
                                    start=False, stop=bank_last)
                    anchors["mm"] = mi
                    sig = sp.tile([128, 4 * BC], F16, tag="sig", name="sig")
                    nc.scalar.activation(sig[:], pa[:], AF.Sigmoid)
                    tg = sp.tile([128, 2 * BC], F16, tag="tg", name="tg")
                    nc.scalar.activation(tg[:], pb[:, 2 * BC:4 * BC], AF.Tanh)
                    sigo = sp.tile([128, 2 * BC], F16, tag="sigo", name="sigo")
                    nc.scalar.activation(sigo[:], pb[:, 0:2 * BC], AF.Sigmoid)
                    v = cp.tile([128, 2 * BC], F32, tag="v", name="v")
                    nc.vector.tensor_tensor(v[:], sig[:, 2 * BC:4 * BC],
                                            st["c"], OP.mult)
                    u = cp.tile([128, 2 * BC], F16, tag="u", name="u")
                    nc.vector.tensor_tensor(u[:], sig[:, 0:2 * BC], tg[:], OP.mult)
                    cn = cp.tile([128, 2 * BC], F32, tag="c", name="c")
                    nc.vector.tensor_tensor(cn[:], u[:], v[:], OP.add)
                    st["sig"], st["sigo"], st["cn"] = sig, sigo, cn

                def tail(t):
                    """h = sig_o * tanh(c)"""
                    sigo, cn = st["sigo"], st["cn"]
                    thc = sp.tile([128, 2 * BC], F16, tag="thc", name="thc")
                    nc.scalar.activation(thc[:], cn[:], AF.Tanh)
                    anchors["h"] = nc.vector.tensor_tensor(
                        hs[:, (t + 1) * HB:(t + 2) * HB],
                        sigo[:], thc[:], OP.mult)
                    st["c"] = cn

                PP = 16                     # steps per pooling piece
                parts = []

                def pool_piece(t0, n=PP):
                    """masked partial sum of h over steps [t0, t0+n)"""
                    mk = pp_pool.tile([128, PP * HB], F16, tag="mk",
                                      name=f"mk{t0}", bufs=2)
                    nc.gpsimd.tensor_tensor(
                        mk[:, :n * HB], hs[:, (t0 + 1) * HB:(t0 + n + 1) * HB],
                        mb[:, t0 * HB:(t0 + n) * HB], OP.mult)
                    # reduce over t as a binary tree of gpsimd adds
                    # (gpsimd tensor_reduce can't reduce the free axis)
                    cur = mk
                    w = n * HB
                    while w > HB:
                        w //= 2
                        nxt = pp_pool.tile([128, w], F32, tag=f"mkr{w}",
                                           name=f"mkr{w}_{t0}", bufs=2)
                        nc.gpsimd.tensor_tensor(
                            nxt[:], cur[:, 0:w], cur[:, w:2 * w], OP.add)
                        cur = nxt
                    parts.append(cur)
                    if len(parts) >= 2:
                        a, b = parts.pop(), parts.pop()
                        s = pp_pool.tile([128, HB], F32, tag="psum",
                                         name=f"ps{t0}", bufs=2)
                        nc.gpsimd.tensor_tensor(s[:], a[:], b[:], OP.add)
                        parts.append(s)

                # ---------------- interleaved schedule
                #
                # Single strictly-ordered work queue, popped only from the
                # head. 'pe' items (gather DMA + transposes, proj matmuls) are
                # issued behind a scheduler fence placed right after front(t),
                # so they fill the long PE-idle stretch of the step without
                # ever being schedulable ahead of the recurrent matmuls. 'dve'
                # items (PSUM->SBUF copies) are issued behind a fence placed
                # after tail(t), so they fill the post-h DVE-idle window and
                # can never head-block a chain op. Popping only from the head
                # keeps issue order = queue order, which the PSUM pool slab
                # cycling (prj/tp tags) relies on.
                from collections import deque
                work = deque()
                for kind, fn in (chunk_work(0) + mb_items(16) + chunk_work(1)):
                    fn()
                work.extend(chunk_work(2))
                work.append(("pe", count_mm))
                work.append(("dve", count_fin))
                next_chunk = 3
                for ci in range(len(chunks)):
                    t0, ns = chunks[ci]
                    if next_chunk < len(chunks):
                        work.extend(chunk_work(next_chunk))
                        next_chunk += 1
                    work.extendleft(reversed(mb_items(t0 + ns)))
                    for t in range(t0, t0 + ns):
                        with tc.high_priority():
                            front(t)
                            tail(t)
                        if t >= PP + PP // 2 and (t - PP // 2) % PP == 0 \
                                and t - PP - PP // 2 < T - PP:
                            pool_piece(t - PP - PP // 2)
                        if t == T - 5:
                            pool_piece(T - PP, PP // 2)
                        for _ in range(4):
                            if work:
                                work.popleft()[1]()
                assert not work, f"{len(work)} work items unissued"
                pool_piece(T - PP // 2, PP // 2)

                # ---------------- tail: pooled -> logits
                while len(parts) > 1:
                    a, b = parts.pop(), parts.pop()
                    s = pp_pool.tile([128, HB], F32, tag="psum",
                                     name=f"fin{len(parts)}", bufs=2)
                    nc.vector.tensor_tensor(s[:], a[:], b[:], OP.add)
                    parts.append(s)
                pooled = parts[0]

                pn = pp_pool.tile([128, HB], F32, tag="pn")
                nc.vector.tensor_tensor(pn[:], pooled[:], cnt_r[:], OP.mult)
                lg = prp.tile([C, BL], F32, tag="prj", name="lg")
                for k in range(2):
                    nc.tensor.matmul(lg[:], wcA[:, k * C:(k + 1) * C],
                                     pn[:, k * BL:(k + 1) * BL],
                                     start=(k == 0), stop=(k == 1))
                ot = pp_pool.tile([C, BL], F32, tag="ot")
                nc.scalar.activation(ot[:], lg[:], AF.Identity, bias=bc_t[:])
                nc.sync.dma_start(out_ap[:], ot[:])

    nc.compile()
    return nc


# ---------------------------------------------------------------- entry

_NC_CACHE = {}


def kernel(**inputs) -> np.ndarray:
    """BiLSTM classifier forward on 8 trn2 NeuronCores.

    Takes the full unsharded inputs (as produced by setup_inputs()), runs
    the SPMD bass kernel on cores 0-7, returns full [64, 3] f32 logits.
    """
    T = 256
    if T not in _NC_CACHE:
        _NC_CACHE[T] = build_nc(T=T)
    nc = _NC_CACHE[T]
    np_inputs = {k: np.asarray(v) for k, v in inputs.items()}
    in_maps = prep_in_maps(T=T, **np_inputs)
    res = run_bass_kernel_spmd(nc, in_maps, list(range(NCORES)))
    return assemble(res.results)
